# revision 1
# baseline (speedup 1.0000x reference)
"""GraphSAGE (3-layer, mean-agg + BN + ReLU) SPMD kernel for trn2 NeuronCores.

Sharding: dst-node shards of n_nodes/n_cores per core. Per core, edges are
sorted by (dst_tile, table_half, src) and padded into 128-edge chunks that are
dst-tile pure and table-half pure (dma_gather uses int16 indices, so source
tables are addressed in two halves, split at 32768). Aggregation per chunk is
a one-hot matmul accumulated in PSUM, feature-major:
    agg_T[din_blk, 128 dst] += gathered[128 e, din_blk].T @ S[128 e, 128 dst]
Layers 1/2 use the z-trick: z = h @ Wl computed per-shard node-major,
AllGathered, gathered by src (mean division commutes with Wl). The bl biases
cancel under BN and are dropped. BN stats are feature-major bn_stats/bn_aggr,
combined across cores with a small AllGather of (mean, var).
"""
import numpy as np
import ml_dtypes
import concourse.bass as bass
import concourse.bacc as bacc
import concourse.tile as tile
from concourse import mybir
from concourse.masks import make_identity
from concourse.library_config import mlp

P = 128
F32 = mybir.dt.float32
BF16 = mybir.dt.bfloat16
I32 = mybir.dt.int32
I16 = mybir.dt.int16
MAX_CALL = 8  # chunks per dma_gather call (<=1024 idxs)


# ---------------------------------------------------------------- host prep
def preprocess(edge_index, n_nodes, n_cores, split_at=32768):
    src = np.asarray(edge_index[0], dtype=np.int64)
    dst = np.asarray(edge_index[1], dtype=np.int64)
    shard = n_nodes // n_cores
    ntiles = (shard + P - 1) // P

    deg = np.bincount(dst, minlength=n_nodes).astype(np.float32)
    rd_full = (1.0 / np.maximum(deg, 1.0)).astype(np.float32)

    core_of = dst // shard
    tile_of = (dst % shard) // P
    half_of = (src >= split_at).astype(np.int64)
    order = np.lexsort((src, half_of, tile_of, core_of))
    src_s, dst_s = src[order], dst[order]

    key = (core_of[order] * ntiles + tile_of[order]) * 2 + half_of[order]
    counts = np.bincount(key, minlength=n_cores * ntiles * 2).reshape(n_cores, ntiles, 2)
    nch = np.ceil(counts / P).astype(np.int64).max(axis=0)  # [ntiles, 2] shared
    ntot = int(nch.sum())

    starts = np.zeros(n_cores * ntiles * 2 + 1, dtype=np.int64)
    np.cumsum(counts.reshape(-1), out=starts[1:])

    pvec = np.arange(P)
    per_core = []
    for c in range(n_cores):
        idx16 = np.zeros((16, ntot * 8), dtype=np.int16)
        dstl = np.full((P, ntot), -1, dtype=np.int32)
        ci = 0
        for t in range(ntiles):
            for h in range(2):
                k = (c * ntiles + t) * 2 + h
                lo, hi = int(starts[k]), int(starts[k + 1])
                e_src = src_s[lo:hi] - h * split_at
                e_dst = dst_s[lo:hi] % shard - t * P
                for cc in range(int(nch[t, h])):
                    a = cc * P
                    sl_src = e_src[a:a + P]
                    sl_dst = e_dst[a:a + P]
                    m = len(sl_src)
                    if m > 0:
                        pv = pvec[:m]
                        idx16[pv % 16, ci * 8 + pv // 16] = sl_src.astype(np.int16)
                        dstl[:m, ci] = sl_dst.astype(np.int32)
                    ci += 1
        assert ci == ntot
        per_core.append({
            "idx16": np.tile(idx16, (8, 1)),
            "dstl": dstl,
            "rd": np.concatenate([
                rd_full[c * shard:(c + 1) * shard],
                np.ones(ntiles * P - shard, np.float32)]),
        })

    meta = {"n_nodes": n_nodes, "n_cores": n_cores, "shard": shard,
            "ntiles": ntiles, "nch": nch, "ntot": ntot, "split_at": split_at}
    return meta, per_core


def _calls_for(n, max_call=MAX_CALL):
    out = []
    n = int(n)
    while n > 0:
        take = min(n, max_call)
        out.append(take)
        n -= take
    return out


# ---------------------------------------------------------------- builder
def build_kernel(meta, dims, eps=1e-5):
    n_cores = meta["n_cores"]
    shard, ntiles, ntot = meta["shard"], meta["ntiles"], meta["ntot"]
    nch = meta["nch"]
    split_at = meta["split_at"]
    n_nodes = meta["n_nodes"]
    d0, d1, d2, d3 = dims
    assert d0 == P
    nb1, nb2, nb3 = d1 // P, d2 // P, d3 // P

    nc = bacc.Bacc(debug=False, num_devices=n_cores)

    xg = nc.declare_dram_parameter("xg", [n_nodes, d0], BF16, isOutput=False)
    x_own_T = nc.declare_dram_parameter("x_own_T", [d0, shard], F32, isOutput=False)
    idx16_d = nc.declare_dram_parameter("idx16", [P, ntot * 8], I16, isOutput=False)
    dstl_d = nc.declare_dram_parameter("dstl", [P, ntot], I32, isOutput=False)
    rd_d = nc.declare_dram_parameter("rd", [ntiles * P], F32, isOutput=False)
    Wl0 = nc.declare_dram_parameter("Wl0", [d0, d1], F32, isOutput=False)
    Wr0 = nc.declare_dram_parameter("Wr0", [d0, d1], F32, isOutput=False)
    Wl1 = nc.declare_dram_parameter("Wl1", [d1, d2], BF16, isOutput=False)
    Wr1 = nc.declare_dram_parameter("Wr1", [d1, d2], BF16, isOutput=False)
    Wl2 = nc.declare_dram_parameter("Wl2", [d2, d3], BF16, isOutput=False)
    Wr2 = nc.declare_dram_parameter("Wr2", [d2, d3], BF16, isOutput=False)
    g_d = [nc.declare_dram_parameter(f"gn{i}", [dims[i + 1]], F32, isOutput=False) for i in range(3)]
    b_d = [nc.declare_dram_parameter(f"bn{i}", [dims[i + 1]], F32, isOutput=False) for i in range(3)]
    yout = nc.declare_dram_parameter("yout", [shard, d3], F32, isOutput=True)

    rg = [list(range(n_cores))]

    # chunk offsets
    chunk0 = np.zeros((ntiles, 2), dtype=np.int64)
    acc = 0
    for t in range(ntiles):
        for h in range(2):
            chunk0[t, h] = acc
            acc += int(nch[t, h])

    def tw(t):
        return min(P, shard - t * P)

    from contextlib import ExitStack
    with tile.TileContext(nc) as tc, ExitStack() as _st:
        pp = _st.enter_context(tc.tile_pool(name="persist", bufs=1))
        sp = _st.enter_context(tc.tile_pool(name="onehot", bufs=2))
        gp = _st.enter_context(tc.tile_pool(name="gath", bufs=3))
        wp = _st.enter_context(tc.tile_pool(name="work", bufs=3))
        smp = _st.enter_context(tc.tile_pool(name="small", bufs=4))
        psA = _st.enter_context(tc.tile_pool(name="psA", bufs=2, space="PSUM"))
        psB = _st.enter_context(tc.tile_pool(name="psB", bufs=2, space="PSUM"))
        psC = _st.enter_context(tc.tile_pool(name="psC", bufs=2, space="PSUM"))
        dp = _st.enter_context(tc.tile_pool(name="dram", bufs=1, space="DRAM"))
        hAB = _st.enter_context(tc.tile_pool(name="hAB", bufs=1))

        nc.gpsimd.load_library(mlp)

        idx_t = pp.tile([P, ntot * 8], I16)
        nc.gpsimd.dma_start(out=idx_t[:], in_=idx16_d[:])
        dstl_f = pp.tile([P, ntot], F32)
        dstl_i = wp.tile([P, ntot], I32, tag="dstli")
        nc.gpsimd.dma_start(out=dstl_i[:], in_=dstl_d[:])
        nc.vector.tensor_copy(out=dstl_f[:], in_=dstl_i[:])
        iota_f = pp.tile([P, P], F32)
        iota_i = wp.tile([P, P], I32, tag="iotai")
        nc.gpsimd.iota(iota_i[:], pattern=[[1, P]], base=0, channel_multiplier=0)
        nc.vector.tensor_copy(out=iota_f[:], in_=iota_i[:])
        ident = pp.tile([P, P], F32)
        make_identity(nc, ident[:])
        eps_t = pp.tile([P, 1], F32)
        nc.vector.memset(eps_t[:], float(eps))

        # bf16 weights (persist)
        wl1 = pp.tile([P, (d1 // P) * d2], BF16)
        nc.sync.dma_start(out=wl1[:].rearrange("p (k n) -> p k n", n=d2), in_=Wl1[:].rearrange("(k p) n -> p k n", p=P))
        wr1 = pp.tile([P, (d1 // P) * d2], BF16)
        nc.sync.dma_start(out=wr1[:].rearrange("p (k n) -> p k n", n=d2), in_=Wr1[:].rearrange("(k p) n -> p k n", p=P))
        wl2 = pp.tile([P, (d2 // P) * d3], BF16)
        nc.sync.dma_start(out=wl2[:].rearrange("p (k n) -> p k n", n=d3), in_=Wl2[:].rearrange("(k p) n -> p k n", p=P))
        wr2 = pp.tile([P, (d2 // P) * d3], BF16)
        nc.sync.dma_start(out=wr2[:].rearrange("p (k n) -> p k n", n=d3), in_=Wr2[:].rearrange("(k p) n -> p k n", p=P))

        # internal DRAM
        z1_sh = dp.tile([shard, d2], BF16)
        z1_full = dp.tile([n_cores * shard, d2], BF16)
        z2_sh = dp.tile([shard, d3], BF16)
        z2_full = dp.tile([n_cores * shard, d3], BF16)
        st_sh = [dp.tile([P, 2 * n], F32, tag=f"stsh{i}", name=f"stsh{i}") for i, n in enumerate((nb1, nb2, nb3))]
        st_full = [dp.tile([n_cores * P, 2 * n], F32, tag=f"stfl{i}", name=f"stfl{i}") for i, n in enumerate((nb1, nb2, nb3))]

        # ---------------- helpers
        def build_onehot(t):
            nch_t = int(nch[t, 0] + nch[t, 1])
            c0 = int(chunk0[t, 0])
            S = sp.tile([P, nch_t, P], BF16, tag="S")
            nc.vector.tensor_tensor(
                out=S[:],
                in0=dstl_f[:, c0:c0 + nch_t].unsqueeze(2).to_broadcast([P, nch_t, P]),
                in1=iota_f[:].unsqueeze(1).to_broadcast([P, nch_t, P]),
                op=mybir.AluOpType.is_equal,
            )
            return S, c0

        def do_gathers(t, d_in, tab_lo, tab_hi):
            """Returns list of (gtile, rel_chunk, ncall)."""
            c0 = int(chunk0[t, 0])
            gts = []
            for h, tab in ((0, tab_lo), (1, tab_hi)):
                ci = int(chunk0[t, h])
                for ncall in _calls_for(nch[t, h]):
                    g = gp.tile([P, MAX_CALL, d_in], BF16, tag="g")
                    nc.gpsimd.dma_gather(
                        g[:, :ncall, :], tab,
                        idx_t[:, ci * 8:(ci + ncall) * 8],
                        ncall * P, ncall * P, d_in,
                    )
                    gts.append((g, ci - c0, ncall))
                    ci += ncall
            return gts

        def agg_block(S, gts, j, nch_t):
            """One feature block of the aggregate: PSUM [P, P] over all chunks."""
            ps = psA.tile([P, P], F32, tag="agg")
            done = 0
            for g, rel, ncall in gts:
                for cc in range(ncall):
                    nc.tensor.matmul(
                        ps[:],
                        lhsT=g[:, cc, j * P:(j + 1) * P],
                        rhs=S[:, rel + cc, :],
                        start=(done == 0), stop=(done == nch_t - 1),
                    )
                    done += 1
            assert done == nch_t
            return ps

        def rd_bcast(t):
            rdb = smp.tile([P, P], F32, tag="rdb")
            nc.sync.dma_start(
                out=rdb[:], in_=rd_d[t * P:(t + 1) * P].partition_broadcast(P))
            return rdb

        def bn_finalize(layer, stats, nbo, n_sb):
            stg = smp.tile([P, 2 * nbo], F32, tag=f"stg{layer}")
            for j in range(nbo):
                mv = smp.tile([P, 2], F32, tag="mv")
                nc.vector.bn_aggr(out=mv[:], in_=stats[j][:])
                nc.vector.tensor_copy(out=stg[:, 2 * j:2 * j + 2], in_=mv[:])
            nc.sync.dma_start(out=st_sh[layer][:], in_=stg[:])
            nc.gpsimd.collective_compute(
                "AllGather", mybir.AluOpType.bypass,
                ins=[st_sh[layer].opt()], outs=[st_full[layer].opt()],
                replica_groups=rg)
            stall = smp.tile([P, n_cores, 2 * nbo], F32, tag=f"stall{layer}")
            nc.sync.dma_start(
                out=stall[:], in_=st_full[layer][:].rearrange("(c p) s -> p c s", p=P))
            scales, biases = [], []
            for j in range(nbo):
                m_acc = smp.tile([P, 1], F32, tag="macc")
                s_acc = smp.tile([P, 1], F32, tag="sacc")
                nc.vector.memset(m_acc[:], 0.0)
                nc.vector.memset(s_acc[:], 0.0)
                for c in range(n_cores):
                    mc = stall[:, c, 2 * j:2 * j + 1]
                    vc = stall[:, c, 2 * j + 1:2 * j + 2]
                    nc.vector.tensor_add(out=m_acc[:], in0=m_acc[:], in1=mc)
                    t1 = smp.tile([P, 1], F32, tag="t1")
                    nc.vector.tensor_mul(out=t1[:], in0=mc, in1=mc)
                    nc.vector.tensor_add(out=t1[:], in0=t1[:], in1=vc)
                    nc.vector.tensor_add(out=s_acc[:], in0=s_acc[:], in1=t1[:])
                nc.scalar.mul(m_acc[:], m_acc[:], 1.0 / n_cores)
                nc.scalar.mul(s_acc[:], s_acc[:], 1.0 / n_cores)
                t2 = smp.tile([P, 1], F32, tag="t2")
                nc.vector.tensor_mul(out=t2[:], in0=m_acc[:], in1=m_acc[:])
                var = smp.tile([P, 1], F32, tag="var")
                nc.vector.tensor_tensor(out=var[:], in0=s_acc[:], in1=t2[:],
                                        op=mybir.AluOpType.subtract)
                rs = smp.tile([P, 1], F32, tag="rs")
                nc.scalar.activation(out=rs[:], in_=var[:],
                                     func=mybir.ActivationFunctionType.Sqrt,
                                     bias=eps_t[:], scale=1.0)
                nc.vector.reciprocal(out=rs[:], in_=rs[:])
                gt = smp.tile([P, 1], F32, tag="gt")
                nc.sync.dma_start(out=gt[:], in_=g_d[layer][j * P:(j + 1) * P].unsqueeze(1))
                bt = smp.tile([P, 1], F32, tag="bt")
                nc.sync.dma_start(out=bt[:], in_=b_d[layer][j * P:(j + 1) * P].unsqueeze(1))
                sc = n_sb.tile([P, 1], F32, tag=f"sc{layer}_{j}", name=f"sc{layer}_{j}")
                nc.vector.tensor_mul(out=sc[:], in0=gt[:], in1=rs[:])
                bi = n_sb.tile([P, 1], F32, tag=f"bi{layer}_{j}", name=f"bi{layer}_{j}")
                nc.vector.tensor_mul(out=bi[:], in0=m_acc[:], in1=sc[:])
                nc.vector.tensor_tensor(out=bi[:], in0=bt[:], in1=bi[:],
                                        op=mybir.AluOpType.subtract)
                scales.append(sc)
                biases.append(bi)
            return scales, biases

        def bn_apply(store, scales, biases, nbo, out_dtype_note=None):
            for j in range(nbo):
                for t in range(ntiles):
                    w = tw(t)
                    nc.scalar.activation(
                        out=store[j][:, t * P:t * P + w],
                        in_=store[j][:, t * P:t * P + w],
                        func=mybir.ActivationFunctionType.Relu,
                        bias=biases[j][:], scale=scales[j][:])

        # =============== LAYER 0 ===============
        hA = [hAB.tile([P, ntiles * P], BF16, tag=f"hA{j}", name=f"hA{j}") for j in range(nb1)]
        hB = [hAB.tile([P, ntiles * P], BF16, tag=f"hB{j}", name=f"hB{j}") for j in range(nb2)]

        l0_cm = tc.tile_pool(name="l0", bufs=1)
        l0p = l0_cm.__enter__()
        xoT = l0p.tile([P, shard], F32)
        nc.sync.dma_start(out=xoT[:], in_=x_own_T[:])
        wl0 = l0p.tile([P, d1], F32)
        nc.sync.dma_start(out=wl0[:], in_=Wl0[:])
        wr0 = l0p.tile([P, d1], F32)
        nc.sync.dma_start(out=wr0[:], in_=Wr0[:])
        stats0 = [l0p.tile([P, ntiles, 6], F32, tag=f"st0_{j}", name=f"st0_{j}") for j in range(nb1)]

        for t in range(ntiles):
            w = tw(t)
            nch_t = int(nch[t, 0] + nch[t, 1])
            S, _ = build_onehot(t)
            gts = do_gathers(t, d0, xg[:], xg[split_at:, :])
            agg = agg_block(S, gts, 0, nch_t)  # [P(din), P(dst)] psum
            rdb = rd_bcast(t)
            mean0 = wp.tile([P, P], F32, tag="mean")
            nc.vector.tensor_mul(out=mean0[:], in0=agg[:], in1=rdb[:])
            for j in range(nb1):
                ph = psB.tile([P, P], F32, tag="mm")
                nc.tensor.matmul(ph[:, :w], lhsT=wl0[:, j * P:(j + 1) * P],
                                 rhs=mean0[:, :w], start=True, stop=False)
                nc.tensor.matmul(ph[:, :w], lhsT=wr0[:, j * P:(j + 1) * P],
                                 rhs=xoT[:, t * P:t * P + w], start=False, stop=True)
                nc.vector.bn_stats(out=stats0[j][:, t, :], in_=ph[:, :w])
                nc.scalar.copy(out=hA[j][:, t * P:t * P + w], in_=ph[:, :w])

        sc0, bi0 = bn_finalize(0, stats0, nb1, pp)
        bn_apply(hA, sc0, bi0, nb1)   # hA now holds h1 (bf16)
        l0_cm.__exit__(None, None, None)

        # =============== z1 + AllGather ===============
        for t in range(ntiles):
            w = tw(t)
            pz = psC.tile([P, d2], F32, tag="z")
            for k in range(d1 // P):
                nc.tensor.matmul(pz[:w, :], lhsT=hA[k][:, t * P:t * P + w],
                                 rhs=wl1[:, k * d2:(k + 1) * d2],
                                 start=(k == 0), stop=(k == d1 // P - 1))
            zs = wp.tile([P, d2], BF16, tag="zs")
            nc.scalar.copy(out=zs[:w, :], in_=pz[:w, :])
            nc.sync.dma_start(out=z1_sh[t * P:t * P + w, :], in_=zs[:w, :])
        nc.gpsimd.collective_compute(
            "AllGather", mybir.AluOpType.bypass,
            ins=[z1_sh.opt()], outs=[z1_full.opt()], replica_groups=rg)

        # =============== LAYER 1 ===============
        l1_cm = tc.tile_pool(name="l1", bufs=1)
        l1p = l1_cm.__enter__()
        stats1 = [l1p.tile([P, ntiles, 6], F32, tag=f"st1_{j}", name=f"st1_{j}") for j in range(nb2)]

        for t in range(ntiles):
            w = tw(t)
            nch_t = int(nch[t, 0] + nch[t, 1])
            S, _ = build_onehot(t)
            gts = do_gathers(t, d2, z1_full[:], z1_full[split_at:, :])
            rdb = rd_bcast(t)
            for j in range(nb2):
                agg = agg_block(S, gts, j, nch_t)
                pw = psB.tile([P, P], F32, tag="mm")
                for k in range(d1 // P):
                    nc.tensor.matmul(
                        pw[:, :w],
                        lhsT=wr1[:, k * d2 + j * P:k * d2 + (j + 1) * P],
                        rhs=hA[k][:, t * P:t * P + w],
                        start=(k == 0), stop=(k == d1 // P - 1))
                mean1 = wp.tile([P, P], F32, tag="mean")
                nc.vector.tensor_mul(out=mean1[:], in0=agg[:], in1=rdb[:])
                raw = wp.tile([P, P], F32, tag="raw")
                nc.vector.tensor_add(out=raw[:, :w], in0=mean1[:, :w], in1=pw[:, :w])
                nc.vector.bn_stats(out=stats1[j][:, t, :], in_=raw[:, :w])
                nc.scalar.copy(out=hB[j][:, t * P:t * P + w], in_=raw[:, :w])

        sc1, bi1 = bn_finalize(1, stats1, nb2, pp)
        bn_apply(hB, sc1, bi1, nb2)   # hB = h2 (bf16)
        l1_cm.__exit__(None, None, None)

        # =============== z2 + AllGather ===============
        for t in range(ntiles):
            w = tw(t)
            pz = psC.tile([P, max(d3, 1)], F32, tag="z")
            for k in range(d2 // P):
                nc.tensor.matmul(pz[:w, :], lhsT=hB[k][:, t * P:t * P + w],
                                 rhs=wl2[:, k * d3:(k + 1) * d3],
                                 start=(k == 0), stop=(k == d2 // P - 1))
            zs = wp.tile([P, d3], BF16, tag="zs2")
            nc.scalar.copy(out=zs[:w, :], in_=pz[:w, :])
            nc.sync.dma_start(out=z2_sh[t * P:t * P + w, :], in_=zs[:w, :])
        nc.gpsimd.collective_compute(
            "AllGather", mybir.AluOpType.bypass,
            ins=[z2_sh.opt()], outs=[z2_full.opt()], replica_groups=rg)

        # =============== LAYER 2 ===============
        l2_cm = tc.tile_pool(name="l2", bufs=1)
        l2p = l2_cm.__enter__()
        rawC = [l2p.tile([P, ntiles * P], F32, tag=f"rawC{j}", name=f"rawC{j}") for j in range(nb3)]
        stats2 = [l2p.tile([P, ntiles, 6], F32, tag=f"st2_{j}", name=f"st2_{j}") for j in range(nb3)]

        for t in range(ntiles):
            w = tw(t)
            nch_t = int(nch[t, 0] + nch[t, 1])
            S, _ = build_onehot(t)
            gts = do_gathers(t, d3, z2_full[:], z2_full[split_at:, :])
            rdb = rd_bcast(t)
            for j in range(nb3):
                agg = agg_block(S, gts, j, nch_t)
                pw = psB.tile([P, P], F32, tag="mm")
                for k in range(d2 // P):
                    nc.tensor.matmul(
                        pw[:, :w],
                        lhsT=wr2[:, k * d3 + j * P:k * d3 + (j + 1) * P],
                        rhs=hB[k][:, t * P:t * P + w],
                        start=(k == 0), stop=(k == d2 // P - 1))
                mean2 = wp.tile([P, P], F32, tag="mean")
                nc.vector.tensor_mul(out=mean2[:], in0=agg[:], in1=rdb[:])
                raw = wp.tile([P, P], F32, tag="raw")
                nc.vector.tensor_add(out=raw[:, :w], in0=mean2[:, :w], in1=pw[:, :w])
                nc.vector.bn_stats(out=stats2[j][:, t, :], in_=raw[:, :w])
                nc.scalar.copy(out=rawC[j][:, t * P:t * P + w], in_=raw[:, :w])

        sc2, bi2 = bn_finalize(2, stats2, nb3, pp)
        bn_apply(rawC, sc2, bi2, nb3)  # rawC = h3 (fp32, feature-major)

        # transpose to node-major and write out
        for t in range(ntiles):
            w = tw(t)
            for j in range(nb3):
                pt = psC.tile([P, P], F32, tag="z")
                nc.tensor.transpose(out=pt[:], in_=rawC[j][:, t * P:(t + 1) * P],
                                    identity=ident[:])
                ot = wp.tile([P, P], F32, tag="ot")
                nc.scalar.copy(out=ot[:w, :], in_=pt[:w, :])
                nc.sync.dma_start(out=yout[t * P:t * P + w, j * P:(j + 1) * P],
                                    in_=ot[:w, :])

        l2_cm.__exit__(None, None, None)

    nc.compile()
    return nc


# ---------------------------------------------------------------- top level
def make_in_maps(x, edge_index, weights, meta, per_core):
    """weights: dict with Wl0..Wl2, Wr0..Wr2, g0..g2, b0..b2 (numpy fp32)."""
    n_cores, shard = meta["n_cores"], meta["shard"]
    bf = lambda a: np.asarray(a, dtype=ml_dtypes.bfloat16)
    f32 = lambda a: np.ascontiguousarray(np.asarray(a, dtype=np.float32))
    shared = {
        "xg": bf(x),
        "Wl0": f32(weights["Wl0"]), "Wr0": f32(weights["Wr0"]),
        "Wl1": bf(weights["Wl1"]), "Wr1": bf(weights["Wr1"]),
        "Wl2": bf(weights["Wl2"]), "Wr2": bf(weights["Wr2"]),
        "gn0": f32(weights["g0"]), "bn0": f32(weights["b0"]),
        "gn1": f32(weights["g1"]), "bn1": f32(weights["b1"]),
        "gn2": f32(weights["g2"]), "bn2": f32(weights["b2"]),
    }
    in_maps = []
    for c in range(n_cores):
        m = dict(shared)
        m["x_own_T"] = f32(np.asarray(x[c * shard:(c + 1) * shard]).T)
        m["idx16"] = per_core[c]["idx16"]
        m["dstl"] = per_core[c]["dstl"]
        m["rd"] = per_core[c]["rd"]
        in_maps.append(m)
    return in_maps


# ============================================================ entry point
_N_NODES = 50000
_DIMS = [128, 512, 256, 128]
_N_CORES = 8
_EPS = 1e-5


def kernel(x, edge_index, Wl0, bl0, Wr0, g0, b0, Wl1, bl1, Wr1, g1, b1,
           Wl2, bl2, Wr2, g2, b2):
    """Full-input GraphSAGE forward on 8 trn2 NeuronCores. bl* cancel under
    BatchNorm and are unused."""
    from concourse.bass_utils import run_bass_kernel_spmd
    x = np.asarray(x, dtype=np.float32)
    edge_index = np.asarray(edge_index)
    meta, per_core = preprocess(edge_index, _N_NODES, _N_CORES)
    nc = build_kernel(meta, _DIMS, eps=_EPS)
    weights = {
        "Wl0": np.asarray(Wl0), "Wr0": np.asarray(Wr0),
        "Wl1": np.asarray(Wl1), "Wr1": np.asarray(Wr1),
        "Wl2": np.asarray(Wl2), "Wr2": np.asarray(Wr2),
        "g0": np.asarray(g0), "b0": np.asarray(b0),
        "g1": np.asarray(g1), "b1": np.asarray(b1),
        "g2": np.asarray(g2), "b2": np.asarray(b2),
    }
    in_maps = make_in_maps(x, edge_index, weights, meta, per_core)
    res = run_bass_kernel_spmd(nc, in_maps, list(range(_N_CORES)))
    out = np.concatenate([res.results[c]["yout"] for c in range(_N_CORES)], axis=0)
    return out.astype(np.float32)



# revision 2
# speedup vs baseline: 3.6249x; 3.6249x over previous
"""GraphSAGE (3-layer, mean-agg + BN + ReLU) SPMD kernel for trn2 NeuronCores.

Sharding: dst-node shards of n_nodes/n_cores per core. Per core, edges are
sorted by (dst_tile, table_half, src) and padded into 128-edge chunks that are
dst-tile pure and table-half pure (dma_gather uses int16 indices, so source
tables are addressed in two halves, split at 32768). Aggregation per chunk is
a one-hot matmul accumulated in PSUM, feature-major:
    agg_T[din_blk, 128 dst] += gathered[128 e, din_blk].T @ S[128 e, 128 dst]
Layers 1/2 use the z-trick: z = h @ Wl computed per-shard node-major,
AllGathered, gathered by src (mean division commutes with Wl). The bl biases
cancel under BN and are dropped. BN stats are feature-major bn_stats/bn_aggr,
combined across cores with a small AllGather of (mean, var).
"""
import numpy as np
import ml_dtypes
import concourse.bass as bass
import concourse.bacc as bacc
import concourse.tile as tile
from concourse import mybir
from concourse.masks import make_identity
from concourse.library_config import mlp

P = 128
F32 = mybir.dt.float32
BF16 = mybir.dt.bfloat16
I32 = mybir.dt.int32
I16 = mybir.dt.int16
MAX_CALL = 8  # chunks per dma_gather call (<=1024 idxs)
EXP = dict(max_call=8, gq=4, single_packet=True, gwidth=None, gpbufs=8,
           fp8z1=True, negpad=False, ymaj=True, deep=True, l0bf=True,
           band=0)
FP8 = mybir.dt.float8e3
Z1SCALE = 4.0


# ---------------------------------------------------------------- host prep
def preprocess(edge_index, n_nodes, n_cores, split_at=32768):
    src = np.asarray(edge_index[0], dtype=np.int64)
    dst = np.asarray(edge_index[1], dtype=np.int64)
    shard = n_nodes // n_cores
    ntiles = (shard + P - 1) // P

    deg = np.bincount(dst, minlength=n_nodes).astype(np.float32)
    rd_full = (1.0 / np.maximum(deg, 1.0)).astype(np.float32)

    core_of = dst // shard
    tile_of = (dst % shard) // P
    half_of = (src >= split_at).astype(np.int64)
    order = np.lexsort((src, half_of, tile_of, core_of))
    src_s, dst_s = src[order], dst[order]

    key = (core_of[order] * ntiles + tile_of[order]) * 2 + half_of[order]
    counts = np.bincount(key, minlength=n_cores * ntiles * 2).reshape(n_cores, ntiles, 2)
    nch = np.ceil(counts / P).astype(np.int64).max(axis=0)  # [ntiles, 2] shared
    ntot = int(nch.sum())

    starts = np.zeros(n_cores * ntiles * 2 + 1, dtype=np.int64)
    np.cumsum(counts.reshape(-1), out=starts[1:])

    pvec = np.arange(P)
    B = int(EXP.get("band", 0))
    bands = None
    tile_ranges = None
    order_chunks = None
    if B:
        # storage order (band, half, tile, k): calls pack across tiles per half
        order_chunks = []
        tile_ranges = np.zeros((ntiles, 4), dtype=np.int64)  # a0 n0 a1 n1
        bands = []
        mc = int(EXP["max_call"])
        ci = 0
        for b0 in range(0, ntiles, B):
            tiles_b = list(range(b0, min(ntiles, b0 + B)))
            calls = []
            for h in (0, 1):
                h_start = ci
                for t in tiles_b:
                    tile_ranges[t, 2 * h] = ci
                    tile_ranges[t, 2 * h + 1] = nch[t, h]
                    for k in range(int(nch[t, h])):
                        order_chunks.append((t, h, k))
                        ci += 1
                s = h_start
                while s < ci:
                    take = min(mc, ci - s)
                    calls.append((h, s, take))
                    s += take
            bands.append({"tiles": tiles_b, "calls": calls})
        assert ci == ntot

    per_core = []
    pad_idx = -1 if EXP.get("negpad") else 0
    for c in range(n_cores):
        idx16 = np.full((16, ntot * 8), pad_idx, dtype=np.int16)
        dstl = np.full((P, ntot), -1, dtype=np.int32)
        if B:
            for ci, (t, h, cc) in enumerate(order_chunks):
                k = (c * ntiles + t) * 2 + h
                lo, hi = int(starts[k]), int(starts[k + 1])
                a = cc * P
                sl_src = src_s[lo:hi][a:a + P] - h * split_at
                sl_dst = dst_s[lo:hi][a:a + P] % shard - t * P
                m = len(sl_src)
                if m > 0:
                    pv = pvec[:m]
                    idx16[pv % 16, ci * 8 + pv // 16] = sl_src.astype(np.int16)
                    dstl[:m, ci] = sl_dst.astype(np.int32)
        else:
            ci = 0
            for t in range(ntiles):
                for h in range(2):
                    k = (c * ntiles + t) * 2 + h
                    lo, hi = int(starts[k]), int(starts[k + 1])
                    e_src = src_s[lo:hi] - h * split_at
                    e_dst = dst_s[lo:hi] % shard - t * P
                    for cc in range(int(nch[t, h])):
                        a = cc * P
                        sl_src = e_src[a:a + P]
                        sl_dst = e_dst[a:a + P]
                        m = len(sl_src)
                        if m > 0:
                            pv = pvec[:m]
                            idx16[pv % 16, ci * 8 + pv // 16] = sl_src.astype(np.int16)
                            dstl[:m, ci] = sl_dst.astype(np.int32)
                        ci += 1
            assert ci == ntot
        per_core.append({
            "idx16": np.tile(idx16, (8, 1)),
            "dstl": dstl,
            "rd": np.concatenate([
                rd_full[c * shard:(c + 1) * shard],
                np.ones(ntiles * P - shard, np.float32)]),
        })

    meta = {"n_nodes": n_nodes, "n_cores": n_cores, "shard": shard,
            "ntiles": ntiles, "nch": nch, "ntot": ntot, "split_at": split_at,
            "bands": bands, "tile_ranges": tile_ranges}
    return meta, per_core


def _calls_for(n, max_call=None):
    if max_call is None:
        max_call = MAX_CALL
    out = []
    n = int(n)
    while n > 0:
        take = min(n, max_call)
        out.append(take)
        n -= take
    return out


# ---------------------------------------------------------------- builder
def build_kernel(meta, dims, eps=1e-5, ablate=()):
    ablate = set(ablate)
    n_cores = meta["n_cores"]
    shard, ntiles, ntot = meta["shard"], meta["ntiles"], meta["ntot"]
    nch = meta["nch"]
    split_at = meta["split_at"]
    n_nodes = meta["n_nodes"]
    d0, d1, d2, d3 = dims
    assert d0 == P
    nb1, nb2, nb3 = d1 // P, d2 // P, d3 // P

    nc = bacc.Bacc(debug=False, num_devices=n_cores,
                   num_swdge_queues=EXP["gq"])

    l0dt = BF16 if EXP["l0bf"] else F32
    xg = nc.declare_dram_parameter("xg", [n_nodes, d0], BF16, isOutput=False)
    x_own_T = nc.declare_dram_parameter("x_own_T", [d0, shard], l0dt, isOutput=False)
    idx16_d = nc.declare_dram_parameter("idx16", [P, ntot * 8], I16, isOutput=False)
    dstl_d = nc.declare_dram_parameter("dstl", [P, ntot], I32, isOutput=False)
    rd_d = nc.declare_dram_parameter("rd", [ntiles * P], F32, isOutput=False)
    rd8_d = nc.declare_dram_parameter("rd8", [ntiles * P], F32, isOutput=False) \
        if EXP["fp8z1"] else None
    Wl0 = nc.declare_dram_parameter("Wl0", [d0, d1], l0dt, isOutput=False)
    Wr0 = nc.declare_dram_parameter("Wr0", [d0, d1], l0dt, isOutput=False)
    Wl1 = nc.declare_dram_parameter("Wl1", [d1, d2], BF16, isOutput=False)
    Wr1 = nc.declare_dram_parameter("Wr1", [d1, d2], BF16, isOutput=False)
    Wl2 = nc.declare_dram_parameter("Wl2", [d2, d3], BF16, isOutput=False)
    Wr2 = nc.declare_dram_parameter("Wr2", [d2, d3], BF16, isOutput=False)
    g_d = [nc.declare_dram_parameter(f"gn{i}", [dims[i + 1]], F32, isOutput=False) for i in range(3)]
    b_d = [nc.declare_dram_parameter(f"bn{i}", [dims[i + 1]], F32, isOutput=False) for i in range(3)]
    if EXP["ymaj"]:
        yout = nc.declare_dram_parameter("yout", [d3, shard], F32, isOutput=True)
    else:
        yout = nc.declare_dram_parameter("yout", [shard, d3], F32, isOutput=True)

    rg = [list(range(n_cores))]

    # chunk offsets
    chunk0 = np.zeros((ntiles, 2), dtype=np.int64)
    acc = 0
    for t in range(ntiles):
        for h in range(2):
            chunk0[t, h] = acc
            acc += int(nch[t, h])

    def tw(t):
        return min(P, shard - t * P)

    from contextlib import ExitStack
    with tile.TileContext(nc) as tc, ExitStack() as _st:
        deep = EXP["deep"]
        pp = _st.enter_context(tc.tile_pool(name="persist", bufs=1))
        sp = _st.enter_context(tc.tile_pool(name="onehot", bufs=3 if deep else 2))
        gp = _st.enter_context(tc.tile_pool(name="gath", bufs=EXP["gpbufs"]))
        wp = _st.enter_context(tc.tile_pool(name="work", bufs=4 if deep else 3))
        smp = _st.enter_context(tc.tile_pool(name="small", bufs=6 if deep else 4))
        psA = _st.enter_context(tc.tile_pool(name="psA", bufs=3 if deep else 2, space="PSUM"))
        psB = _st.enter_context(tc.tile_pool(name="psB", bufs=3 if deep else 2, space="PSUM"))
        psC = _st.enter_context(tc.tile_pool(name="psC", bufs=2, space="PSUM"))
        dp = _st.enter_context(tc.tile_pool(name="dram", bufs=1, space="DRAM"))
        hAB = _st.enter_context(tc.tile_pool(name="hAB", bufs=1))

        nc.gpsimd.load_library(mlp)

        idx_t = pp.tile([P, ntot * 8], I16)
        nc.gpsimd.dma_start(out=idx_t[:], in_=idx16_d[:])
        dstl_f = pp.tile([P, ntot], BF16)
        dstl_i = wp.tile([P, ntot], I32, tag="dstli")
        nc.gpsimd.dma_start(out=dstl_i[:], in_=dstl_d[:])
        nc.vector.tensor_copy(out=dstl_f[:], in_=dstl_i[:])
        iota_f = pp.tile([P, P], BF16)
        iota_i = wp.tile([P, P], I32, tag="iotai")
        nc.gpsimd.iota(iota_i[:], pattern=[[1, P]], base=0, channel_multiplier=0)
        nc.vector.tensor_copy(out=iota_f[:], in_=iota_i[:])
        ident = pp.tile([P, P], F32)
        make_identity(nc, ident[:])
        eps_t = pp.tile([P, 1], F32)
        nc.vector.memset(eps_t[:], float(eps))

        if EXP["negpad"]:
            # trailing -1 idxs leave g rows unwritten; make sure every pool
            # slot holds finite values so 0*garbage stays 0 in the agg matmul
            for _ in range(EXP["gpbufs"]):
                gz = gp.tile([P, EXP["max_call"], 256], BF16, tag="g")
                nc.vector.memset(gz[:], 0.0)

        # bf16 weights (persist)
        wl1 = pp.tile([P, (d1 // P) * d2], BF16)
        nc.sync.dma_start(out=wl1[:].rearrange("p (k n) -> p k n", n=d2), in_=Wl1[:].rearrange("(k p) n -> p k n", p=P))
        wr1 = pp.tile([P, (d1 // P) * d2], BF16)
        nc.sync.dma_start(out=wr1[:].rearrange("p (k n) -> p k n", n=d2), in_=Wr1[:].rearrange("(k p) n -> p k n", p=P))
        wl2 = pp.tile([P, (d2 // P) * d3], BF16)
        nc.sync.dma_start(out=wl2[:].rearrange("p (k n) -> p k n", n=d3), in_=Wl2[:].rearrange("(k p) n -> p k n", p=P))
        wr2 = pp.tile([P, (d2 // P) * d3], BF16)
        nc.sync.dma_start(out=wr2[:].rearrange("p (k n) -> p k n", n=d3), in_=Wr2[:].rearrange("(k p) n -> p k n", p=P))

        # internal DRAM
        z1dt = FP8 if EXP["fp8z1"] else BF16
        z1_sh = dp.tile([shard, d2], z1dt)
        z1_full = dp.tile([n_cores * shard, d2], z1dt)
        z2_sh = dp.tile([shard, d3], BF16)
        z2_full = dp.tile([n_cores * shard, d3], BF16)
        st_sh = [dp.tile([P, 2 * n], F32, tag=f"stsh{i}", name=f"stsh{i}") for i, n in enumerate((nb1, nb2, nb3))]
        st_full = [dp.tile([n_cores * P, 2 * n], F32, tag=f"stfl{i}", name=f"stfl{i}") for i, n in enumerate((nb1, nb2, nb3))]

        bands = meta.get("bands")
        tile_ranges = meta.get("tile_ranges")

        # ---------------- helpers
        def build_onehot(t):
            nch_t = int(nch[t, 0] + nch[t, 1])
            S = sp.tile([P, nch_t, P], BF16, tag="S")
            if bands is not None:
                a0, n0, a1, n1 = (int(v) for v in tile_ranges[t])
                assert n0 + n1 == nch_t
                for (a, n, off) in ((a0, n0, 0), (a1, n1, n0)):
                    if n:
                        nc.vector.tensor_tensor(
                            out=S[:, off:off + n, :],
                            in0=dstl_f[:, a:a + n].unsqueeze(2).to_broadcast([P, n, P]),
                            in1=iota_f[:].unsqueeze(1).to_broadcast([P, n, P]),
                            op=mybir.AluOpType.is_equal,
                        )
                return S, None
            c0 = int(chunk0[t, 0])
            if "onehot" in ablate:
                nc.vector.memset(S[:, 0, :], 0.0)
            else:
                nc.vector.tensor_tensor(
                    out=S[:],
                    in0=dstl_f[:, c0:c0 + nch_t].unsqueeze(2).to_broadcast([P, nch_t, P]),
                    in1=iota_f[:].unsqueeze(1).to_broadcast([P, nch_t, P]),
                    op=mybir.AluOpType.is_equal,
                )
            return S, c0

        gcall_n = [0]

        def do_gathers(t, d_in, tab_lo, tab_hi, dt=BF16):
            """Returns list of (gtile, rel_chunk, ncall)."""
            mc = EXP["max_call"]
            gw = EXP["gwidth"] or d_in
            c0 = int(chunk0[t, 0])
            gts = []
            for h, tab in ((0, tab_lo), (1, tab_hi)):
                ci = int(chunk0[t, h])
                for ncall in _calls_for(nch[t, h], mc):
                    g = gp.tile([P, mc, gw], dt, tag="g")
                    if "gather" in ablate:
                        nc.vector.memset(g[:, 0, :P], 0.0)
                    else:
                        nc.gpsimd.dma_gather(
                            g[:, :ncall, :], tab[:, :gw] if gw != d_in else tab,
                            idx_t[:, ci * 8:(ci + ncall) * 8],
                            ncall * P, ncall * P, gw,
                            elem_step=d_in,
                            single_packet=EXP["single_packet"],
                            queue_num=gcall_n[0] % EXP["gq"],
                        )
                        gcall_n[0] += 1
                    gts.append((g, ci - c0, ncall))
                    ci += ncall
            return gts

        def band_gathers(band, d_in, tab_lo, tab_hi, dt, gmap):
            mc = EXP["max_call"]
            # interleave the two halves' calls so every tile's chunks land early
            cl0 = [c for c in band["calls"] if c[0] == 0]
            cl1 = [c for c in band["calls"] if c[0] == 1]
            merged, i, j = [], 0, 0
            while i < len(cl0) or j < len(cl1):
                if j >= len(cl1) or (i < len(cl0) and i * max(len(cl1), 1) <= j * max(len(cl0), 1)):
                    merged.append(cl0[i]); i += 1
                else:
                    merged.append(cl1[j]); j += 1
            for (h, s, take) in merged:
                tab = tab_lo if h == 0 else tab_hi
                g = gp.tile([P, mc, d_in], dt, tag="g")
                nc.gpsimd.dma_gather(
                    g[:, :take, :], tab,
                    idx_t[:, s * 8:(s + take) * 8],
                    take * P, take * P, d_in,
                    single_packet=EXP["single_packet"],
                    queue_num=gcall_n[0] % EXP["gq"],
                )
                gcall_n[0] += 1
                for k in range(take):
                    gmap[s + k] = (g, k)

        def tile_chunks(t, gmap):
            a0, n0, a1, n1 = (int(v) for v in tile_ranges[t])
            out = []
            for k in range(n0):
                g, sl = gmap[a0 + k]
                out.append((g, sl, k))
            for k in range(n1):
                g, sl = gmap[a1 + k]
                out.append((g, sl, n0 + k))
            return out

        def agg_block_b(S, chunks, j):
            ps = psA.tile([P, P], F32, tag="agg")
            n = len(chunks)
            for i, (g, sl, rel) in enumerate(chunks):
                nc.tensor.matmul(ps[:], lhsT=g[:, sl, j * P:(j + 1) * P],
                                 rhs=S[:, rel, :],
                                 start=(i == 0), stop=(i == n - 1))
            return ps

        def agg_block(S, gts, j, nch_t):
            """One feature block of the aggregate: PSUM [P, P] over all chunks."""
            ps = psA.tile([P, P], F32, tag="agg")
            if "aggmm" in ablate:
                g, rel, ncall = gts[0]
                nc.tensor.matmul(ps[:], lhsT=g[:, 0, j * P:(j + 1) * P],
                                 rhs=S[:, rel, :], start=True, stop=True)
                return ps
            done = 0
            for g, rel, ncall in gts:
                for cc in range(ncall):
                    nc.tensor.matmul(
                        ps[:],
                        lhsT=g[:, cc, j * P:(j + 1) * P],
                        rhs=S[:, rel + cc, :],
                        start=(done == 0), stop=(done == nch_t - 1),
                    )
                    done += 1
            assert done == nch_t
            return ps

        def layer_tiles(d_in, tab_lo, tab_hi, dt, body):
            """Iterate tiles with gathers; body(t, S, agg_fn)."""
            if bands is not None:
                for band in bands:
                    gmap = {}
                    band_gathers(band, d_in, tab_lo, tab_hi, dt, gmap)
                    for t in band["tiles"]:
                        S, _ = build_onehot(t)
                        chunks = tile_chunks(t, gmap)
                        body(t, S, lambda j, S=S, ch=chunks: agg_block_b(S, ch, j))
            else:
                for t in range(ntiles):
                    nch_t = int(nch[t, 0] + nch[t, 1])
                    S, _ = build_onehot(t)
                    gts = do_gathers(t, d_in, tab_lo, tab_hi, dt=dt)
                    body(t, S,
                         lambda j, S=S, g=gts, n=nch_t: agg_block(S, g, j, n))

        def rd_bcast(t, src=None):
            rdb = smp.tile([P, P], F32, tag="rdb")
            s = rd_d if src is None else src
            nc.sync.dma_start(
                out=rdb[:], in_=s[t * P:(t + 1) * P].partition_broadcast(P))
            return rdb

        def bn_finalize(layer, stats, nbo, n_sb):
            stg = smp.tile([P, 2 * nbo], F32, tag=f"stg{layer}")
            for j in range(nbo):
                mv = smp.tile([P, 2], F32, tag="mv")
                nc.vector.bn_aggr(out=mv[:], in_=stats[j][:])
                nc.vector.tensor_copy(out=stg[:, 2 * j:2 * j + 2], in_=mv[:])
            nc.sync.dma_start(out=st_sh[layer][:], in_=stg[:])
            nc.gpsimd.collective_compute(
                "AllGather", mybir.AluOpType.bypass,
                ins=[st_sh[layer].opt()], outs=[st_full[layer].opt()],
                replica_groups=rg)
            stall = smp.tile([P, n_cores, 2 * nbo], F32, tag=f"stall{layer}")
            nc.sync.dma_start(
                out=stall[:], in_=st_full[layer][:].rearrange("(c p) s -> p c s", p=P))
            scales, biases = [], []
            for j in range(nbo):
                m_acc = smp.tile([P, 1], F32, tag="macc")
                s_acc = smp.tile([P, 1], F32, tag="sacc")
                nc.vector.memset(m_acc[:], 0.0)
                nc.vector.memset(s_acc[:], 0.0)
                for c in range(n_cores):
                    mc = stall[:, c, 2 * j:2 * j + 1]
                    vc = stall[:, c, 2 * j + 1:2 * j + 2]
                    nc.vector.tensor_add(out=m_acc[:], in0=m_acc[:], in1=mc)
                    t1 = smp.tile([P, 1], F32, tag="t1")
                    nc.vector.tensor_mul(out=t1[:], in0=mc, in1=mc)
                    nc.vector.tensor_add(out=t1[:], in0=t1[:], in1=vc)
                    nc.vector.tensor_add(out=s_acc[:], in0=s_acc[:], in1=t1[:])
                nc.scalar.mul(m_acc[:], m_acc[:], 1.0 / n_cores)
                nc.scalar.mul(s_acc[:], s_acc[:], 1.0 / n_cores)
                t2 = smp.tile([P, 1], F32, tag="t2")
                nc.vector.tensor_mul(out=t2[:], in0=m_acc[:], in1=m_acc[:])
                var = smp.tile([P, 1], F32, tag="var")
                nc.vector.tensor_tensor(out=var[:], in0=s_acc[:], in1=t2[:],
                                        op=mybir.AluOpType.subtract)
                rs = smp.tile([P, 1], F32, tag="rs")
                nc.scalar.activation(out=rs[:], in_=var[:],
                                     func=mybir.ActivationFunctionType.Sqrt,
                                     bias=eps_t[:], scale=1.0)
                nc.vector.reciprocal(out=rs[:], in_=rs[:])
                gt = smp.tile([P, 1], F32, tag="gt")
                nc.sync.dma_start(out=gt[:], in_=g_d[layer][j * P:(j + 1) * P].unsqueeze(1))
                bt = smp.tile([P, 1], F32, tag="bt")
                nc.sync.dma_start(out=bt[:], in_=b_d[layer][j * P:(j + 1) * P].unsqueeze(1))
                sc = n_sb.tile([P, 1], F32, tag=f"sc{layer}_{j}", name=f"sc{layer}_{j}")
                nc.vector.tensor_mul(out=sc[:], in0=gt[:], in1=rs[:])
                bi = n_sb.tile([P, 1], F32, tag=f"bi{layer}_{j}", name=f"bi{layer}_{j}")
                nc.vector.tensor_mul(out=bi[:], in0=m_acc[:], in1=sc[:])
                nc.vector.tensor_tensor(out=bi[:], in0=bt[:], in1=bi[:],
                                        op=mybir.AluOpType.subtract)
                scales.append(sc)
                biases.append(bi)
            return scales, biases

        def bn_apply(store, scales, biases, nbo, out_dtype_note=None):
            for j in range(nbo):
                for t in range(ntiles):
                    w = tw(t)
                    nc.scalar.activation(
                        out=store[j][:, t * P:t * P + w],
                        in_=store[j][:, t * P:t * P + w],
                        func=mybir.ActivationFunctionType.Relu,
                        bias=biases[j][:], scale=scales[j][:])

        # =============== LAYER 0 ===============
        hA = [hAB.tile([P, ntiles * P], BF16, tag=f"hA{j}", name=f"hA{j}") for j in range(nb1)]
        hB = [hAB.tile([P, ntiles * P], BF16, tag=f"hB{j}", name=f"hB{j}") for j in range(nb2)]

        l0_cm = tc.tile_pool(name="l0", bufs=1)
        l0p = l0_cm.__enter__()
        xoT = l0p.tile([P, shard], l0dt)
        nc.sync.dma_start(out=xoT[:], in_=x_own_T[:])
        wl0 = l0p.tile([P, d1], l0dt)
        nc.sync.dma_start(out=wl0[:], in_=Wl0[:])
        wr0 = l0p.tile([P, d1], l0dt)
        nc.sync.dma_start(out=wr0[:], in_=Wr0[:])
        stats0 = [l0p.tile([P, ntiles, 6], F32, tag=f"st0_{j}", name=f"st0_{j}") for j in range(nb1)]

        def l0_body(t, S, agg_fn):
            w = tw(t)
            agg = agg_fn(0)  # [P(din), P(dst)] psum
            rdb = rd_bcast(t)
            mean0 = wp.tile([P, P], l0dt, tag="mean")
            nc.vector.tensor_mul(out=mean0[:], in0=agg[:], in1=rdb[:])
            for j in range(nb1):
                ph = psB.tile([P, P], F32, tag="mm")
                if "dense" in ablate:
                    nc.tensor.matmul(ph[:, :w], lhsT=wl0[:, j * P:(j + 1) * P],
                                     rhs=mean0[:, :w], start=True, stop=True)
                else:
                    nc.tensor.matmul(ph[:, :w], lhsT=wl0[:, j * P:(j + 1) * P],
                                     rhs=mean0[:, :w], start=True, stop=False)
                    nc.tensor.matmul(ph[:, :w], lhsT=wr0[:, j * P:(j + 1) * P],
                                     rhs=xoT[:, t * P:t * P + w], start=False, stop=True)
                nc.vector.bn_stats(out=stats0[j][:, t, :], in_=ph[:, :w])
                nc.scalar.copy(out=hA[j][:, t * P:t * P + w], in_=ph[:, :w])

        layer_tiles(d0, xg[:], xg[split_at:, :], BF16, l0_body)

        sc0, bi0 = bn_finalize(0, stats0, nb1, pp)
        bn_apply(hA, sc0, bi0, nb1)   # hA now holds h1 (bf16)
        l0_cm.__exit__(None, None, None)

        # =============== z1 + AllGather ===============
        for t in range(ntiles):
            w = tw(t)
            pz = psC.tile([P, d2], F32, tag="z")
            for k in range(d1 // P):
                nc.tensor.matmul(pz[:w, :], lhsT=hA[k][:, t * P:t * P + w],
                                 rhs=wl1[:, k * d2:(k + 1) * d2],
                                 start=(k == 0), stop=(k == d1 // P - 1))
            zs = wp.tile([P, d2], z1dt, tag="zs")
            if EXP["fp8z1"]:
                nc.scalar.mul(zs[:w, :], pz[:w, :], Z1SCALE)
            else:
                nc.scalar.copy(out=zs[:w, :], in_=pz[:w, :])
            nc.sync.dma_start(out=z1_sh[t * P:t * P + w, :], in_=zs[:w, :])
        nc.gpsimd.collective_compute(
            "AllGather", mybir.AluOpType.bypass,
            ins=[z1_sh.opt()], outs=[z1_full.opt()], replica_groups=rg)

        # =============== LAYER 1 ===============
        l1_cm = tc.tile_pool(name="l1", bufs=1)
        l1p = l1_cm.__enter__()
        stats1 = [l1p.tile([P, ntiles, 6], F32, tag=f"st1_{j}", name=f"st1_{j}") for j in range(nb2)]

        def l1_body(t, S, agg_fn):
            w = tw(t)
            rdb = rd_bcast(t, rd8_d)
            for j in range(nb2):
                agg = agg_fn(j)
                pw = psB.tile([P, P], F32, tag="mm")
                kr1 = 1 if "dense" in ablate else d1 // P
                for k in range(kr1):
                    nc.tensor.matmul(
                        pw[:, :w],
                        lhsT=wr1[:, k * d2 + j * P:k * d2 + (j + 1) * P],
                        rhs=hA[k][:, t * P:t * P + w],
                        start=(k == 0), stop=(k == kr1 - 1))
                mean1 = wp.tile([P, P], F32, tag="mean")
                nc.vector.tensor_mul(out=mean1[:], in0=agg[:], in1=rdb[:])
                raw = wp.tile([P, P], F32, tag="raw")
                nc.vector.tensor_add(out=raw[:, :w], in0=mean1[:, :w], in1=pw[:, :w])
                nc.vector.bn_stats(out=stats1[j][:, t, :], in_=raw[:, :w])
                nc.scalar.copy(out=hB[j][:, t * P:t * P + w], in_=raw[:, :w])

        layer_tiles(d2, z1_full[:], z1_full[split_at:, :], z1dt, l1_body)

        sc1, bi1 = bn_finalize(1, stats1, nb2, pp)
        bn_apply(hB, sc1, bi1, nb2)   # hB = h2 (bf16)
        l1_cm.__exit__(None, None, None)

        # =============== z2 + AllGather ===============
        for t in range(ntiles):
            w = tw(t)
            pz = psC.tile([P, max(d3, 1)], F32, tag="z")
            for k in range(d2 // P):
                nc.tensor.matmul(pz[:w, :], lhsT=hB[k][:, t * P:t * P + w],
                                 rhs=wl2[:, k * d3:(k + 1) * d3],
                                 start=(k == 0), stop=(k == d2 // P - 1))
            zs = wp.tile([P, d3], BF16, tag="zs2")
            nc.scalar.copy(out=zs[:w, :], in_=pz[:w, :])
            nc.sync.dma_start(out=z2_sh[t * P:t * P + w, :], in_=zs[:w, :])
        nc.gpsimd.collective_compute(
            "AllGather", mybir.AluOpType.bypass,
            ins=[z2_sh.opt()], outs=[z2_full.opt()], replica_groups=rg)

        # =============== LAYER 2 ===============
        l2_cm = tc.tile_pool(name="l2", bufs=1)
        l2p = l2_cm.__enter__()
        rawC = [l2p.tile([P, ntiles * P], F32, tag=f"rawC{j}", name=f"rawC{j}") for j in range(nb3)]
        stats2 = [l2p.tile([P, ntiles, 6], F32, tag=f"st2_{j}", name=f"st2_{j}") for j in range(nb3)]

        def l2_body(t, S, agg_fn):
            w = tw(t)
            rdb = rd_bcast(t)
            for j in range(nb3):
                agg = agg_fn(j)
                pw = psB.tile([P, P], F32, tag="mm")
                kr2 = 1 if "dense" in ablate else d2 // P
                for k in range(kr2):
                    nc.tensor.matmul(
                        pw[:, :w],
                        lhsT=wr2[:, k * d3 + j * P:k * d3 + (j + 1) * P],
                        rhs=hB[k][:, t * P:t * P + w],
                        start=(k == 0), stop=(k == kr2 - 1))
                mean2 = wp.tile([P, P], F32, tag="mean")
                nc.vector.tensor_mul(out=mean2[:], in0=agg[:], in1=rdb[:])
                raw = wp.tile([P, P], F32, tag="raw")
                nc.vector.tensor_add(out=raw[:, :w], in0=mean2[:, :w], in1=pw[:, :w])
                nc.vector.bn_stats(out=stats2[j][:, t, :], in_=raw[:, :w])
                nc.scalar.copy(out=rawC[j][:, t * P:t * P + w], in_=raw[:, :w])

        layer_tiles(d3, z2_full[:], z2_full[split_at:, :], BF16, l2_body)

        sc2, bi2 = bn_finalize(2, stats2, nb3, pp)
        bn_apply(rawC, sc2, bi2, nb3)  # rawC = h3 (fp32, feature-major)

        # write out: feature-major directly, or transpose to node-major
        if EXP["ymaj"]:
            for j in range(nb3):
                nc.sync.dma_start(out=yout[j * P:(j + 1) * P, :],
                                  in_=rawC[j][:, :shard])
        else:
            for t in range(ntiles):
                w = tw(t)
                for j in range(nb3):
                    pt = psC.tile([P, P], F32, tag="z")
                    nc.tensor.transpose(out=pt[:], in_=rawC[j][:, t * P:(t + 1) * P],
                                        identity=ident[:])
                    ot = wp.tile([P, P], F32, tag="ot")
                    nc.scalar.copy(out=ot[:w, :], in_=pt[:w, :])
                    nc.sync.dma_start(out=yout[t * P:t * P + w, j * P:(j + 1) * P],
                                      in_=ot[:w, :])

        l2_cm.__exit__(None, None, None)

    nc.compile()
    return nc


# ---------------------------------------------------------------- top level
def make_in_maps(x, edge_index, weights, meta, per_core):
    """weights: dict with Wl0..Wl2, Wr0..Wr2, g0..g2, b0..b2 (numpy fp32)."""
    n_cores, shard = meta["n_cores"], meta["shard"]
    bf = lambda a: np.asarray(a, dtype=ml_dtypes.bfloat16)
    f32 = lambda a: np.ascontiguousarray(np.asarray(a, dtype=np.float32))
    l0c = bf if EXP["l0bf"] else f32
    shared = {
        "xg": bf(x),
        "Wl0": l0c(weights["Wl0"]), "Wr0": l0c(weights["Wr0"]),
        "Wl1": bf(weights["Wl1"]), "Wr1": bf(weights["Wr1"]),
        "Wl2": bf(weights["Wl2"]), "Wr2": bf(weights["Wr2"]),
        "gn0": f32(weights["g0"]), "bn0": f32(weights["b0"]),
        "gn1": f32(weights["g1"]), "bn1": f32(weights["b1"]),
        "gn2": f32(weights["g2"]), "bn2": f32(weights["b2"]),
    }
    in_maps = []
    for c in range(n_cores):
        m = dict(shared)
        m["x_own_T"] = l0c(np.ascontiguousarray(np.asarray(x[c * shard:(c + 1) * shard]).T))
        m["idx16"] = per_core[c]["idx16"]
        m["dstl"] = per_core[c]["dstl"]
        m["rd"] = per_core[c]["rd"]
        if EXP["fp8z1"]:
            m["rd8"] = (per_core[c]["rd"] / np.float32(Z1SCALE)).astype(np.float32)
        in_maps.append(m)
    return in_maps


# ============================================================ entry point
_N_NODES = 50000
_DIMS = [128, 512, 256, 128]
_N_CORES = 8
_EPS = 1e-5


def kernel(x, edge_index, Wl0, bl0, Wr0, g0, b0, Wl1, bl1, Wr1, g1, b1,
           Wl2, bl2, Wr2, g2, b2):
    """Full-input GraphSAGE forward on 8 trn2 NeuronCores. bl* cancel under
    BatchNorm and are unused."""
    from concourse.bass_utils import run_bass_kernel_spmd
    x = np.asarray(x, dtype=np.float32)
    edge_index = np.asarray(edge_index)
    meta, per_core = preprocess(edge_index, _N_NODES, _N_CORES)
    nc = build_kernel(meta, _DIMS, eps=_EPS)
    weights = {
        "Wl0": np.asarray(Wl0), "Wr0": np.asarray(Wr0),
        "Wl1": np.asarray(Wl1), "Wr1": np.asarray(Wr1),
        "Wl2": np.asarray(Wl2), "Wr2": np.asarray(Wr2),
        "g0": np.asarray(g0), "b0": np.asarray(b0),
        "g1": np.asarray(g1), "b1": np.asarray(b1),
        "g2": np.asarray(g2), "b2": np.asarray(b2),
    }
    in_maps = make_in_maps(x, edge_index, weights, meta, per_core)
    res = run_bass_kernel_spmd(nc, in_maps, list(range(_N_CORES)))
    if EXP["ymaj"]:
        out = np.concatenate(
            [res.results[c]["yout"].T for c in range(_N_CORES)], axis=0)
    else:
        out = np.concatenate([res.results[c]["yout"] for c in range(_N_CORES)], axis=0)
    return out.astype(np.float32)



# revision 3
# speedup vs baseline: 5.7422x; 1.5841x over previous
"""GraphSAGE (3-layer, mean-agg + BN + ReLU) SPMD kernel for trn2 NeuronCores.

Sharding: dst-node shards of n_nodes/n_cores per core. Per core, edges are
sorted by (dst_tile, table_half, src) and padded into 128-edge chunks that are
dst-tile pure and table-half pure (dma_gather uses int16 indices, so source
tables are addressed in two halves, split at 32768). Aggregation per chunk is
a one-hot matmul accumulated in PSUM, feature-major:
    agg_T[din_blk, 128 dst] += gathered[128 e, din_blk].T @ S[128 e, 128 dst]
Layers 1/2 use the z-trick: z = h @ Wl computed per-shard node-major,
AllGathered, gathered by src (mean division commutes with Wl). The bl biases
cancel under BN and are dropped. BN stats are feature-major bn_stats/bn_aggr,
combined across cores with a small AllGather of (mean, var).
"""
import numpy as np
import ml_dtypes
import concourse.bass as bass
import concourse.bacc as bacc
import concourse.tile as tile
from concourse import mybir
from concourse.masks import make_identity
from concourse.library_config import mlp

P = 128
F32 = mybir.dt.float32
BF16 = mybir.dt.bfloat16
I32 = mybir.dt.int32
I16 = mybir.dt.int16
MAX_CALL = 8  # chunks per dma_gather call (<=1024 idxs)
EXP = dict(max_call=8, gq=4, single_packet=True, gwidth=None, gpbufs=10,
           fp8z1=True, negpad=False, ymaj=True, deep=True, l0bf=True,
           band=0)
FP8 = mybir.dt.float8e3
Z1SCALE = 4.0


# ---------------------------------------------------------------- host prep
def preprocess(edge_index, n_nodes, n_cores, split_at=32768):
    src = np.asarray(edge_index[0], dtype=np.int64)
    dst = np.asarray(edge_index[1], dtype=np.int64)
    shard = n_nodes // n_cores
    ntiles = (shard + P - 1) // P

    deg = np.bincount(dst, minlength=n_nodes).astype(np.float32)
    rd_full = (1.0 / np.maximum(deg, 1.0)).astype(np.float32)

    core_of = dst // shard
    tile_of = (dst % shard) // P
    half_of = (src >= split_at).astype(np.int64)
    order = np.lexsort((src, half_of, tile_of, core_of))
    src_s, dst_s = src[order], dst[order]

    key = (core_of[order] * ntiles + tile_of[order]) * 2 + half_of[order]
    counts = np.bincount(key, minlength=n_cores * ntiles * 2).reshape(n_cores, ntiles, 2)
    nch = np.ceil(counts / P).astype(np.int64).max(axis=0)  # [ntiles, 2] shared
    ntot = int(nch.sum())

    starts = np.zeros(n_cores * ntiles * 2 + 1, dtype=np.int64)
    np.cumsum(counts.reshape(-1), out=starts[1:])

    pvec = np.arange(P)
    B = int(EXP.get("band", 0))
    bands = None
    tile_ranges = None
    order_chunks = None
    if B:
        # storage order (band, half, tile, k): calls pack across tiles per half
        order_chunks = []
        tile_ranges = np.zeros((ntiles, 4), dtype=np.int64)  # a0 n0 a1 n1
        bands = []
        mc = int(EXP["max_call"])
        ci = 0
        for b0 in range(0, ntiles, B):
            tiles_b = list(range(b0, min(ntiles, b0 + B)))
            calls = []
            for h in (0, 1):
                h_start = ci
                for t in tiles_b:
                    tile_ranges[t, 2 * h] = ci
                    tile_ranges[t, 2 * h + 1] = nch[t, h]
                    for k in range(int(nch[t, h])):
                        order_chunks.append((t, h, k))
                        ci += 1
                s = h_start
                while s < ci:
                    take = min(mc, ci - s)
                    calls.append((h, s, take))
                    s += take
            bands.append({"tiles": tiles_b, "calls": calls})
        assert ci == ntot

    per_core = []
    pad_idx = -1 if EXP.get("negpad") else 0
    for c in range(n_cores):
        idx16 = np.full((16, ntot * 8), pad_idx, dtype=np.int16)
        dstl = np.full((P, ntot), -1, dtype=np.int32)
        if B:
            for ci, (t, h, cc) in enumerate(order_chunks):
                k = (c * ntiles + t) * 2 + h
                lo, hi = int(starts[k]), int(starts[k + 1])
                a = cc * P
                sl_src = src_s[lo:hi][a:a + P] - h * split_at
                sl_dst = dst_s[lo:hi][a:a + P] % shard - t * P
                m = len(sl_src)
                if m > 0:
                    pv = pvec[:m]
                    idx16[pv % 16, ci * 8 + pv // 16] = sl_src.astype(np.int16)
                    dstl[:m, ci] = sl_dst.astype(np.int32)
        else:
            ci = 0
            for t in range(ntiles):
                for h in range(2):
                    k = (c * ntiles + t) * 2 + h
                    lo, hi = int(starts[k]), int(starts[k + 1])
                    e_src = src_s[lo:hi] - h * split_at
                    e_dst = dst_s[lo:hi] % shard - t * P
                    for cc in range(int(nch[t, h])):
                        a = cc * P
                        sl_src = e_src[a:a + P]
                        sl_dst = e_dst[a:a + P]
                        m = len(sl_src)
                        if m > 0:
                            pv = pvec[:m]
                            idx16[pv % 16, ci * 8 + pv // 16] = sl_src.astype(np.int16)
                            dstl[:m, ci] = sl_dst.astype(np.int32)
                        ci += 1
            assert ci == ntot
        per_core.append({
            "idx16": np.tile(idx16, (8, 1)),
            "dstl": dstl,
            "rd": np.concatenate([
                rd_full[c * shard:(c + 1) * shard],
                np.ones(ntiles * P - shard, np.float32)]),
        })

    meta = {"n_nodes": n_nodes, "n_cores": n_cores, "shard": shard,
            "ntiles": ntiles, "nch": nch, "ntot": ntot, "split_at": split_at,
            "bands": bands, "tile_ranges": tile_ranges}
    return meta, per_core


def _calls_for(n, max_call=None):
    if max_call is None:
        max_call = MAX_CALL
    out = []
    n = int(n)
    while n > 0:
        take = min(n, max_call)
        out.append(take)
        n -= take
    return out


# ---------------------------------------------------------------- builder
def build_kernel(meta, dims, eps=1e-5, ablate=()):
    ablate = set(ablate)
    n_cores = meta["n_cores"]
    shard, ntiles, ntot = meta["shard"], meta["ntiles"], meta["ntot"]
    nch = meta["nch"]
    split_at = meta["split_at"]
    n_nodes = meta["n_nodes"]
    d0, d1, d2, d3 = dims
    assert d0 == P
    nb1, nb2, nb3 = d1 // P, d2 // P, d3 // P

    nc = bacc.Bacc(debug=False, num_devices=n_cores,
                   num_swdge_queues=EXP["gq"])

    l0dt = BF16 if EXP["l0bf"] else F32
    xg = nc.declare_dram_parameter("xg", [n_nodes, d0], BF16, isOutput=False)
    x_own_T = nc.declare_dram_parameter("x_own_T", [d0, shard], l0dt, isOutput=False)
    idx16_d = nc.declare_dram_parameter("idx16", [P, ntot * 8], I16, isOutput=False)
    dstl_d = nc.declare_dram_parameter("dstl", [P, ntot], I32, isOutput=False)
    rd_d = nc.declare_dram_parameter("rd", [ntiles * P], F32, isOutput=False)
    rd8_d = nc.declare_dram_parameter("rd8", [ntiles * P], F32, isOutput=False) \
        if EXP["fp8z1"] else None
    Wl0 = nc.declare_dram_parameter("Wl0", [d0, d1], l0dt, isOutput=False)
    Wr0 = nc.declare_dram_parameter("Wr0", [d0, d1], l0dt, isOutput=False)
    Wl1 = nc.declare_dram_parameter("Wl1", [d1, d2], BF16, isOutput=False)
    Wr1 = nc.declare_dram_parameter("Wr1", [d1, d2], BF16, isOutput=False)
    Wl2 = nc.declare_dram_parameter("Wl2", [d2, d3], BF16, isOutput=False)
    Wr2 = nc.declare_dram_parameter("Wr2", [d2, d3], BF16, isOutput=False)
    g_d = [nc.declare_dram_parameter(f"gn{i}", [dims[i + 1]], F32, isOutput=False) for i in range(3)]
    b_d = [nc.declare_dram_parameter(f"bn{i}", [dims[i + 1]], F32, isOutput=False) for i in range(3)]
    if EXP["ymaj"]:
        yout = nc.declare_dram_parameter("yout", [d3, shard], F32, isOutput=True)
    else:
        yout = nc.declare_dram_parameter("yout", [shard, d3], F32, isOutput=True)

    rg = [list(range(n_cores))]

    # chunk offsets
    chunk0 = np.zeros((ntiles, 2), dtype=np.int64)
    acc = 0
    for t in range(ntiles):
        for h in range(2):
            chunk0[t, h] = acc
            acc += int(nch[t, h])

    def tw(t):
        return min(P, shard - t * P)

    from contextlib import ExitStack
    with tile.TileContext(nc) as tc, ExitStack() as _st:
        deep = EXP["deep"]
        pp = _st.enter_context(tc.tile_pool(name="persist", bufs=1))
        sp = _st.enter_context(tc.tile_pool(name="onehot", bufs=3 if deep else 2))
        gp = _st.enter_context(tc.tile_pool(name="gath", bufs=EXP["gpbufs"]))
        wp = _st.enter_context(tc.tile_pool(name="work", bufs=4 if deep else 3))
        smp = _st.enter_context(tc.tile_pool(name="small", bufs=6 if deep else 4))
        psA = _st.enter_context(tc.tile_pool(name="psA", bufs=3 if deep else 2, space="PSUM"))
        psB = _st.enter_context(tc.tile_pool(name="psB", bufs=3 if deep else 2, space="PSUM"))
        psC = _st.enter_context(tc.tile_pool(name="psC", bufs=2, space="PSUM"))
        dp = _st.enter_context(tc.tile_pool(name="dram", bufs=1, space="DRAM"))
        hAB = _st.enter_context(tc.tile_pool(name="hAB", bufs=1))

        nc.gpsimd.load_library(mlp)

        idx_t = pp.tile([P, ntot * 8], I16)
        nc.gpsimd.dma_start(out=idx_t[:], in_=idx16_d[:])
        dstl_f = pp.tile([P, ntot], BF16)
        dstl_i = wp.tile([P, ntot], I32, tag="dstli")
        nc.gpsimd.dma_start(out=dstl_i[:], in_=dstl_d[:])
        nc.vector.tensor_copy(out=dstl_f[:], in_=dstl_i[:])
        iota_f = pp.tile([P, P], BF16)
        iota_i = wp.tile([P, P], I32, tag="iotai")
        nc.gpsimd.iota(iota_i[:], pattern=[[1, P]], base=0, channel_multiplier=0)
        nc.vector.tensor_copy(out=iota_f[:], in_=iota_i[:])
        ident = pp.tile([P, P], F32)
        make_identity(nc, ident[:])
        eps_t = pp.tile([P, 1], F32)
        nc.vector.memset(eps_t[:], float(eps))

        if EXP["negpad"]:
            # trailing -1 idxs leave g rows unwritten; make sure every pool
            # slot holds finite values so 0*garbage stays 0 in the agg matmul
            for _ in range(EXP["gpbufs"]):
                gz = gp.tile([P, EXP["max_call"], 256], BF16, tag="g")
                nc.vector.memset(gz[:], 0.0)

        # bf16 weights (persist)
        wl1 = pp.tile([P, (d1 // P) * d2], BF16)
        nc.sync.dma_start(out=wl1[:].rearrange("p (k n) -> p k n", n=d2), in_=Wl1[:].rearrange("(k p) n -> p k n", p=P))
        wr1 = pp.tile([P, (d1 // P) * d2], BF16)
        nc.sync.dma_start(out=wr1[:].rearrange("p (k n) -> p k n", n=d2), in_=Wr1[:].rearrange("(k p) n -> p k n", p=P))
        wl2 = pp.tile([P, (d2 // P) * d3], BF16)
        nc.sync.dma_start(out=wl2[:].rearrange("p (k n) -> p k n", n=d3), in_=Wl2[:].rearrange("(k p) n -> p k n", p=P))
        wr2 = pp.tile([P, (d2 // P) * d3], BF16)
        nc.sync.dma_start(out=wr2[:].rearrange("p (k n) -> p k n", n=d3), in_=Wr2[:].rearrange("(k p) n -> p k n", p=P))

        # internal DRAM
        z1dt = FP8 if EXP["fp8z1"] else BF16
        z1_sh = dp.tile([shard, d2], z1dt)
        z1_full = dp.tile([n_cores * shard, d2], z1dt)
        z2_sh = dp.tile([shard, d3], BF16)
        z2_full = dp.tile([n_cores * shard, d3], BF16)
        st_sh = [dp.tile([P, 2 * n], F32, tag=f"stsh{i}", name=f"stsh{i}") for i, n in enumerate((nb1, nb2, nb3))]
        st_full = [dp.tile([n_cores * P, 2 * n], F32, tag=f"stfl{i}", name=f"stfl{i}") for i, n in enumerate((nb1, nb2, nb3))]

        bands = meta.get("bands")
        tile_ranges = meta.get("tile_ranges")

        # ---------------- helpers
        def build_onehot(t):
            nch_t = int(nch[t, 0] + nch[t, 1])
            S = sp.tile([P, nch_t, P], BF16, tag="S")
            if bands is not None:
                a0, n0, a1, n1 = (int(v) for v in tile_ranges[t])
                assert n0 + n1 == nch_t
                for (a, n, off) in ((a0, n0, 0), (a1, n1, n0)):
                    if n:
                        nc.vector.tensor_tensor(
                            out=S[:, off:off + n, :],
                            in0=dstl_f[:, a:a + n].unsqueeze(2).to_broadcast([P, n, P]),
                            in1=iota_f[:].unsqueeze(1).to_broadcast([P, n, P]),
                            op=mybir.AluOpType.is_equal,
                        )
                return S, None
            c0 = int(chunk0[t, 0])
            if "onehot" in ablate:
                nc.vector.memset(S[:, 0, :], 0.0)
            else:
                nc.vector.tensor_tensor(
                    out=S[:],
                    in0=dstl_f[:, c0:c0 + nch_t].unsqueeze(2).to_broadcast([P, nch_t, P]),
                    in1=iota_f[:].unsqueeze(1).to_broadcast([P, nch_t, P]),
                    op=mybir.AluOpType.is_equal,
                )
            return S, c0

        gcall_n = [0]

        def do_gathers(t, d_in, tab_lo, tab_hi, dt=BF16):
            """Returns list of (gtile, rel_chunk, ncall)."""
            mc = EXP["max_call"]
            gw = EXP["gwidth"] or d_in
            c0 = int(chunk0[t, 0])
            gts = []
            for h, tab in ((0, tab_lo), (1, tab_hi)):
                ci = int(chunk0[t, h])
                for ncall in _calls_for(nch[t, h], mc):
                    g = gp.tile([P, mc, gw], dt, tag="g")
                    if "gather" in ablate:
                        nc.vector.memset(g[:, 0, :P], 0.0)
                    else:
                        nc.gpsimd.dma_gather(
                            g[:, :ncall, :], tab[:, :gw] if gw != d_in else tab,
                            idx_t[:, ci * 8:(ci + ncall) * 8],
                            ncall * P, ncall * P, gw,
                            elem_step=d_in,
                            single_packet=EXP["single_packet"],
                            queue_num=gcall_n[0] % EXP["gq"],
                        )
                        gcall_n[0] += 1
                    gts.append((g, ci - c0, ncall))
                    ci += ncall
            return gts

        def band_gathers(band, d_in, tab_lo, tab_hi, dt, gmap):
            mc = EXP["max_call"]
            # interleave the two halves' calls so every tile's chunks land early
            cl0 = [c for c in band["calls"] if c[0] == 0]
            cl1 = [c for c in band["calls"] if c[0] == 1]
            merged, i, j = [], 0, 0
            while i < len(cl0) or j < len(cl1):
                if j >= len(cl1) or (i < len(cl0) and i * max(len(cl1), 1) <= j * max(len(cl0), 1)):
                    merged.append(cl0[i]); i += 1
                else:
                    merged.append(cl1[j]); j += 1
            for (h, s, take) in merged:
                tab = tab_lo if h == 0 else tab_hi
                g = gp.tile([P, mc, d_in], dt, tag="g")
                nc.gpsimd.dma_gather(
                    g[:, :take, :], tab,
                    idx_t[:, s * 8:(s + take) * 8],
                    take * P, take * P, d_in,
                    single_packet=EXP["single_packet"],
                    queue_num=gcall_n[0] % EXP["gq"],
                )
                gcall_n[0] += 1
                for k in range(take):
                    gmap[s + k] = (g, k)

        def tile_chunks(t, gmap):
            a0, n0, a1, n1 = (int(v) for v in tile_ranges[t])
            out = []
            for k in range(n0):
                g, sl = gmap[a0 + k]
                out.append((g, sl, k))
            for k in range(n1):
                g, sl = gmap[a1 + k]
                out.append((g, sl, n0 + k))
            return out

        def agg_block_b(S, chunks, j):
            ps = psA.tile([P, P], F32, tag="agg")
            n = len(chunks)
            for i, (g, sl, rel) in enumerate(chunks):
                nc.tensor.matmul(ps[:], lhsT=g[:, sl, j * P:(j + 1) * P],
                                 rhs=S[:, rel, :],
                                 start=(i == 0), stop=(i == n - 1))
            return ps

        def agg_block(S, gts, j, nch_t):
            """One feature block of the aggregate: PSUM [P, P] over all chunks."""
            ps = psA.tile([P, P], F32, tag="agg")
            if "aggmm" in ablate:
                g, rel, ncall = gts[0]
                nc.tensor.matmul(ps[:], lhsT=g[:, 0, j * P:(j + 1) * P],
                                 rhs=S[:, rel, :], start=True, stop=True)
                return ps
            done = 0
            for g, rel, ncall in gts:
                for cc in range(ncall):
                    nc.tensor.matmul(
                        ps[:],
                        lhsT=g[:, cc, j * P:(j + 1) * P],
                        rhs=S[:, rel + cc, :],
                        start=(done == 0), stop=(done == nch_t - 1),
                    )
                    done += 1
            assert done == nch_t
            return ps

        def layer_tiles(d_in, tab_lo, tab_hi, dt, body):
            """Iterate tiles with gathers; body(t, S, agg_fn)."""
            if bands is not None:
                for band in bands:
                    gmap = {}
                    band_gathers(band, d_in, tab_lo, tab_hi, dt, gmap)
                    for t in band["tiles"]:
                        S, _ = build_onehot(t)
                        chunks = tile_chunks(t, gmap)
                        body(t, S, lambda j, S=S, ch=chunks: agg_block_b(S, ch, j))
            else:
                for t in range(ntiles):
                    nch_t = int(nch[t, 0] + nch[t, 1])
                    S, _ = build_onehot(t)
                    gts = do_gathers(t, d_in, tab_lo, tab_hi, dt=dt)
                    body(t, S,
                         lambda j, S=S, g=gts, n=nch_t: agg_block(S, g, j, n))

        def rd_bcast(t, src=None):
            rdb = smp.tile([P, P], F32, tag="rdb")
            s = rd_d if src is None else src
            nc.sync.dma_start(
                out=rdb[:], in_=s[t * P:(t + 1) * P].partition_broadcast(P))
            return rdb

        def bn_finalize(layer, stats, nbo, n_sb):
            stg = smp.tile([P, 2 * nbo], F32, tag=f"stg{layer}")
            for j in range(nbo):
                mv = smp.tile([P, 2], F32, tag="mv")
                nc.vector.bn_aggr(out=mv[:], in_=stats[j][:])
                nc.vector.tensor_copy(out=stg[:, 2 * j:2 * j + 2], in_=mv[:])
            nc.sync.dma_start(out=st_sh[layer][:], in_=stg[:])
            nc.gpsimd.collective_compute(
                "AllGather", mybir.AluOpType.bypass,
                ins=[st_sh[layer].opt()], outs=[st_full[layer].opt()],
                replica_groups=rg)
            stall = smp.tile([P, n_cores, 2 * nbo], F32, tag=f"stall{layer}")
            nc.sync.dma_start(
                out=stall[:], in_=st_full[layer][:].rearrange("(c p) s -> p c s", p=P))
            scales, biases = [], []
            for j in range(nbo):
                m_acc = smp.tile([P, 1], F32, tag="macc")
                s_acc = smp.tile([P, 1], F32, tag="sacc")
                nc.vector.memset(m_acc[:], 0.0)
                nc.vector.memset(s_acc[:], 0.0)
                for c in range(n_cores):
                    mc = stall[:, c, 2 * j:2 * j + 1]
                    vc = stall[:, c, 2 * j + 1:2 * j + 2]
                    nc.vector.tensor_add(out=m_acc[:], in0=m_acc[:], in1=mc)
                    t1 = smp.tile([P, 1], F32, tag="t1")
                    nc.vector.tensor_mul(out=t1[:], in0=mc, in1=mc)
                    nc.vector.tensor_add(out=t1[:], in0=t1[:], in1=vc)
                    nc.vector.tensor_add(out=s_acc[:], in0=s_acc[:], in1=t1[:])
                nc.scalar.mul(m_acc[:], m_acc[:], 1.0 / n_cores)
                nc.scalar.mul(s_acc[:], s_acc[:], 1.0 / n_cores)
                t2 = smp.tile([P, 1], F32, tag="t2")
                nc.vector.tensor_mul(out=t2[:], in0=m_acc[:], in1=m_acc[:])
                var = smp.tile([P, 1], F32, tag="var")
                nc.vector.tensor_tensor(out=var[:], in0=s_acc[:], in1=t2[:],
                                        op=mybir.AluOpType.subtract)
                rs = smp.tile([P, 1], F32, tag="rs")
                nc.scalar.activation(out=rs[:], in_=var[:],
                                     func=mybir.ActivationFunctionType.Sqrt,
                                     bias=eps_t[:], scale=1.0)
                nc.vector.reciprocal(out=rs[:], in_=rs[:])
                gt = smp.tile([P, 1], F32, tag="gt")
                nc.sync.dma_start(out=gt[:], in_=g_d[layer][j * P:(j + 1) * P].unsqueeze(1))
                bt = smp.tile([P, 1], F32, tag="bt")
                nc.sync.dma_start(out=bt[:], in_=b_d[layer][j * P:(j + 1) * P].unsqueeze(1))
                sc = n_sb.tile([P, 1], F32, tag=f"sc{layer}_{j}", name=f"sc{layer}_{j}")
                nc.vector.tensor_mul(out=sc[:], in0=gt[:], in1=rs[:])
                bi = n_sb.tile([P, 1], F32, tag=f"bi{layer}_{j}", name=f"bi{layer}_{j}")
                nc.vector.tensor_mul(out=bi[:], in0=m_acc[:], in1=sc[:])
                nc.vector.tensor_tensor(out=bi[:], in0=bt[:], in1=bi[:],
                                        op=mybir.AluOpType.subtract)
                scales.append(sc)
                biases.append(bi)
            return scales, biases

        def bn_apply(store, scales, biases, nbo, out_dtype_note=None):
            for j in range(nbo):
                for t in range(ntiles):
                    w = tw(t)
                    nc.scalar.activation(
                        out=store[j][:, t * P:t * P + w],
                        in_=store[j][:, t * P:t * P + w],
                        func=mybir.ActivationFunctionType.Relu,
                        bias=biases[j][:], scale=scales[j][:])

        # =============== LAYER 0 ===============
        hA = [hAB.tile([P, ntiles * P], BF16, tag=f"hA{j}", name=f"hA{j}") for j in range(nb1)]
        hB = [hAB.tile([P, ntiles * P], BF16, tag=f"hB{j}", name=f"hB{j}") for j in range(nb2)]

        l0_cm = tc.tile_pool(name="l0", bufs=1)
        l0p = l0_cm.__enter__()
        xoT = l0p.tile([P, shard], l0dt)
        nc.sync.dma_start(out=xoT[:], in_=x_own_T[:])
        wl0 = l0p.tile([P, d1], l0dt)
        nc.sync.dma_start(out=wl0[:], in_=Wl0[:])
        wr0 = l0p.tile([P, d1], l0dt)
        nc.sync.dma_start(out=wr0[:], in_=Wr0[:])
        stats0 = [l0p.tile([P, ntiles, 6], F32, tag=f"st0_{j}", name=f"st0_{j}") for j in range(nb1)]

        def l0_body(t, S, agg_fn):
            w = tw(t)
            agg = agg_fn(0)  # [P(din), P(dst)] psum
            rdb = rd_bcast(t)
            mean0 = wp.tile([P, P], l0dt, tag="mean")
            nc.vector.tensor_mul(out=mean0[:], in0=agg[:], in1=rdb[:])
            for j in range(nb1):
                ph = psB.tile([P, P], F32, tag="mm")
                if "dense" in ablate:
                    nc.tensor.matmul(ph[:, :w], lhsT=wl0[:, j * P:(j + 1) * P],
                                     rhs=mean0[:, :w], start=True, stop=True)
                else:
                    nc.tensor.matmul(ph[:, :w], lhsT=wl0[:, j * P:(j + 1) * P],
                                     rhs=mean0[:, :w], start=True, stop=False)
                    nc.tensor.matmul(ph[:, :w], lhsT=wr0[:, j * P:(j + 1) * P],
                                     rhs=xoT[:, t * P:t * P + w], start=False, stop=True)
                nc.vector.bn_stats(out=stats0[j][:, t, :], in_=ph[:, :w])
                nc.scalar.copy(out=hA[j][:, t * P:t * P + w], in_=ph[:, :w])

        layer_tiles(d0, xg[:], xg[split_at:, :], BF16, l0_body)

        sc0, bi0 = bn_finalize(0, stats0, nb1, pp)
        bn_apply(hA, sc0, bi0, nb1)   # hA now holds h1 (bf16)
        l0_cm.__exit__(None, None, None)

        # =============== z1 + AllGather ===============
        for t in range(ntiles):
            w = tw(t)
            pz = psC.tile([P, d2], F32, tag="z")
            for k in range(d1 // P):
                nc.tensor.matmul(pz[:w, :], lhsT=hA[k][:, t * P:t * P + w],
                                 rhs=wl1[:, k * d2:(k + 1) * d2],
                                 start=(k == 0), stop=(k == d1 // P - 1))
            zs = wp.tile([P, d2], z1dt, tag="zs")
            if EXP["fp8z1"]:
                nc.scalar.mul(zs[:w, :], pz[:w, :], Z1SCALE)
            else:
                nc.scalar.copy(out=zs[:w, :], in_=pz[:w, :])
            nc.sync.dma_start(out=z1_sh[t * P:t * P + w, :], in_=zs[:w, :])
        nc.gpsimd.collective_compute(
            "AllGather", mybir.AluOpType.bypass,
            ins=[z1_sh.opt()], outs=[z1_full.opt()], replica_groups=rg)

        # =============== LAYER 1 ===============
        l1_cm = tc.tile_pool(name="l1", bufs=1)
        l1p = l1_cm.__enter__()
        stats1 = [l1p.tile([P, ntiles, 6], F32, tag=f"st1_{j}", name=f"st1_{j}") for j in range(nb2)]

        def l1_body(t, S, agg_fn):
            w = tw(t)
            rdb = rd_bcast(t, rd8_d)
            for j in range(nb2):
                agg = agg_fn(j)
                pw = psB.tile([P, P], F32, tag="mm")
                kr1 = 1 if "dense" in ablate else d1 // P
                for k in range(kr1):
                    nc.tensor.matmul(
                        pw[:, :w],
                        lhsT=wr1[:, k * d2 + j * P:k * d2 + (j + 1) * P],
                        rhs=hA[k][:, t * P:t * P + w],
                        start=(k == 0), stop=(k == kr1 - 1))
                mean1 = wp.tile([P, P], F32, tag="mean")
                nc.vector.tensor_mul(out=mean1[:], in0=agg[:], in1=rdb[:])
                raw = wp.tile([P, P], F32, tag="raw")
                nc.vector.tensor_add(out=raw[:, :w], in0=mean1[:, :w], in1=pw[:, :w])
                nc.vector.bn_stats(out=stats1[j][:, t, :], in_=raw[:, :w])
                nc.scalar.copy(out=hB[j][:, t * P:t * P + w], in_=raw[:, :w])

        layer_tiles(d2, z1_full[:], z1_full[split_at:, :], z1dt, l1_body)

        sc1, bi1 = bn_finalize(1, stats1, nb2, pp)
        bn_apply(hB, sc1, bi1, nb2)   # hB = h2 (bf16)
        l1_cm.__exit__(None, None, None)

        # =============== z2 + AllGather ===============
        for t in range(ntiles):
            w = tw(t)
            pz = psC.tile([P, max(d3, 1)], F32, tag="z")
            for k in range(d2 // P):
                nc.tensor.matmul(pz[:w, :], lhsT=hB[k][:, t * P:t * P + w],
                                 rhs=wl2[:, k * d3:(k + 1) * d3],
                                 start=(k == 0), stop=(k == d2 // P - 1))
            zs = wp.tile([P, d3], BF16, tag="zs2")
            nc.scalar.copy(out=zs[:w, :], in_=pz[:w, :])
            nc.sync.dma_start(out=z2_sh[t * P:t * P + w, :], in_=zs[:w, :])
        nc.gpsimd.collective_compute(
            "AllGather", mybir.AluOpType.bypass,
            ins=[z2_sh.opt()], outs=[z2_full.opt()], replica_groups=rg)

        # =============== LAYER 2 ===============
        l2_cm = tc.tile_pool(name="l2", bufs=1)
        l2p = l2_cm.__enter__()
        rawC = [l2p.tile([P, ntiles * P], F32, tag=f"rawC{j}", name=f"rawC{j}") for j in range(nb3)]
        stats2 = [l2p.tile([P, ntiles, 6], F32, tag=f"st2_{j}", name=f"st2_{j}") for j in range(nb3)]

        def l2_body(t, S, agg_fn):
            w = tw(t)
            rdb = rd_bcast(t)
            for j in range(nb3):
                agg = agg_fn(j)
                pw = psB.tile([P, P], F32, tag="mm")
                kr2 = 1 if "dense" in ablate else d2 // P
                for k in range(kr2):
                    nc.tensor.matmul(
                        pw[:, :w],
                        lhsT=wr2[:, k * d3 + j * P:k * d3 + (j + 1) * P],
                        rhs=hB[k][:, t * P:t * P + w],
                        start=(k == 0), stop=(k == kr2 - 1))
                mean2 = wp.tile([P, P], F32, tag="mean")
                nc.vector.tensor_mul(out=mean2[:], in0=agg[:], in1=rdb[:])
                raw = wp.tile([P, P], F32, tag="raw")
                nc.vector.tensor_add(out=raw[:, :w], in0=mean2[:, :w], in1=pw[:, :w])
                nc.vector.bn_stats(out=stats2[j][:, t, :], in_=raw[:, :w])
                nc.scalar.copy(out=rawC[j][:, t * P:t * P + w], in_=raw[:, :w])

        layer_tiles(d3, z2_full[:], z2_full[split_at:, :], BF16, l2_body)

        sc2, bi2 = bn_finalize(2, stats2, nb3, pp)
        bn_apply(rawC, sc2, bi2, nb3)  # rawC = h3 (fp32, feature-major)

        # write out: feature-major directly, or transpose to node-major
        if EXP["ymaj"]:
            for j in range(nb3):
                nc.sync.dma_start(out=yout[j * P:(j + 1) * P, :],
                                  in_=rawC[j][:, :shard])
        else:
            for t in range(ntiles):
                w = tw(t)
                for j in range(nb3):
                    pt = psC.tile([P, P], F32, tag="z")
                    nc.tensor.transpose(out=pt[:], in_=rawC[j][:, t * P:(t + 1) * P],
                                        identity=ident[:])
                    ot = wp.tile([P, P], F32, tag="ot")
                    nc.scalar.copy(out=ot[:w, :], in_=pt[:w, :])
                    nc.sync.dma_start(out=yout[t * P:t * P + w, j * P:(j + 1) * P],
                                      in_=ot[:w, :])

        l2_cm.__exit__(None, None, None)

    nc.compile()
    return nc


# ---------------------------------------------------------------- top level
def make_in_maps(x, edge_index, weights, meta, per_core):
    """weights: dict with Wl0..Wl2, Wr0..Wr2, g0..g2, b0..b2 (numpy fp32)."""
    n_cores, shard = meta["n_cores"], meta["shard"]
    bf = lambda a: np.asarray(a, dtype=ml_dtypes.bfloat16)
    f32 = lambda a: np.ascontiguousarray(np.asarray(a, dtype=np.float32))
    l0c = bf if EXP["l0bf"] else f32
    shared = {
        "xg": bf(x),
        "Wl0": l0c(weights["Wl0"]), "Wr0": l0c(weights["Wr0"]),
        "Wl1": bf(weights["Wl1"]), "Wr1": bf(weights["Wr1"]),
        "Wl2": bf(weights["Wl2"]), "Wr2": bf(weights["Wr2"]),
        "gn0": f32(weights["g0"]), "bn0": f32(weights["b0"]),
        "gn1": f32(weights["g1"]), "bn1": f32(weights["b1"]),
        "gn2": f32(weights["g2"]), "bn2": f32(weights["b2"]),
    }
    in_maps = []
    for c in range(n_cores):
        m = dict(shared)
        m["x_own_T"] = l0c(np.ascontiguousarray(np.asarray(x[c * shard:(c + 1) * shard]).T))
        m["idx16"] = per_core[c]["idx16"]
        m["dstl"] = per_core[c]["dstl"]
        m["rd"] = per_core[c]["rd"]
        if EXP["fp8z1"]:
            m["rd8"] = (per_core[c]["rd"] / np.float32(Z1SCALE)).astype(np.float32)
        in_maps.append(m)
    return in_maps


# ============================================================ entry point
_N_NODES = 50000
_DIMS = [128, 512, 256, 128]
_N_CORES = 8
_EPS = 1e-5


def kernel(x, edge_index, Wl0, bl0, Wr0, g0, b0, Wl1, bl1, Wr1, g1, b1,
           Wl2, bl2, Wr2, g2, b2):
    """Full-input GraphSAGE forward on 8 trn2 NeuronCores. bl* cancel under
    BatchNorm and are unused."""
    from concourse.bass_utils import run_bass_kernel_spmd
    x = np.asarray(x, dtype=np.float32)
    edge_index = np.asarray(edge_index)
    meta, per_core = preprocess(edge_index, _N_NODES, _N_CORES)
    nc = build_kernel(meta, _DIMS, eps=_EPS)
    weights = {
        "Wl0": np.asarray(Wl0), "Wr0": np.asarray(Wr0),
        "Wl1": np.asarray(Wl1), "Wr1": np.asarray(Wr1),
        "Wl2": np.asarray(Wl2), "Wr2": np.asarray(Wr2),
        "g0": np.asarray(g0), "b0": np.asarray(b0),
        "g1": np.asarray(g1), "b1": np.asarray(b1),
        "g2": np.asarray(g2), "b2": np.asarray(b2),
    }
    in_maps = make_in_maps(x, edge_index, weights, meta, per_core)
    res = run_bass_kernel_spmd(nc, in_maps, list(range(_N_CORES)))
    if EXP["ymaj"]:
        out = np.concatenate(
            [res.results[c]["yout"].T for c in range(_N_CORES)], axis=0)
    else:
        out = np.concatenate([res.results[c]["yout"] for c in range(_N_CORES)], axis=0)
    return out.astype(np.float32)

